# revision 21
# baseline (speedup 1.0000x reference)
"""Conv3d (8,32,48,48,48) * (64,32,3,3,3) -> (8,64,46,46,46), valid, stride 1.

Strategy: data-parallel over batch (1 image per NeuronCore, 8 cores).
Per core the conv is an implicit GEMM:
  out[co, d, h', w'] = sum_{kd,ci,kh,kw} W[co,ci,kd,kh,kw] * X[ci, d+kd, h'+kh, w'+kw]
- contraction K = (kd, ci) = 3*32 = 96 partitions: a SBUF "window" tile
  holds input planes d..d+2 stacked on partitions (plane-major),
- kh, kw are pure free-dim offsets into the window tile (rows step 48),
  so each output tile is 9 accumulating matmuls into one PSUM bank,
- two output planes per window iteration, col-tiled on the PE
  (even plane -> PE col group h0 / PSUM partitions 0:64,
   odd plane  -> col group h64 / PSUM partitions 64:128) so the two
  46x46-position streams run concurrently at 1 col/cycle each,
- ONE merged PSUM tile [128, n] per chunk -> ONE ScalarE activation
  (identity + bias, f32 PSUM -> fp16 SBUF) drains both planes,
- output staged as fp16 [2*CO, 23*SPP] (row = (parity, co)); one DMA
  per plane-pair; host reinterleaves parities and upcasts to f32.
"""

import functools
import os

import numpy as np

import concourse.bacc as bacc
import concourse.tile as tile
from concourse import mybir
from concourse.bass_utils import run_bass_kernel_spmd

# Problem constants (hardcoded per harness contract)
B = 8
CI = 32
DIN = 48
CO = 64
K = 3
DOUT = DIN - K + 1  # 46
SPP = DOUT * DOUT  # 2116 spatial positions per output plane
PLANE = DIN * DIN  # 2304 elements per (ci, plane)
KP = K * CI  # 96 contraction partitions
NPAIR = DOUT // 2  # 23 plane pairs

# h'-row chunking of a 46x46 output plane into PSUM-bank-sized matmuls
CHUNKS = [(0, 10), (10, 9), (19, 9), (28, 9), (37, 9)]  # (h0, rows) -> N = rows*46
# last pair: finer trailing chunks so the final drain+store tail is shorter
CHUNKS_TAIL = [(0, 10), (10, 9), (19, 9), (28, 9), (37, 5), (42, 4)]


F32 = mybir.dt.float32
F16 = mybir.dt.float16

MODE = os.environ.get("CONV_MM_DT", "fp16")
MM_DT = mybir.dt.bfloat16 if MODE == "bf16" else F16


def _pack_mm(a):
    """Host-side cast of a float32 array to the matmul operand format."""
    if MODE == "bf16":
        import ml_dtypes

        return np.ascontiguousarray(a.astype(ml_dtypes.bfloat16))
    return np.ascontiguousarray(a.astype(np.float16))


@functools.lru_cache(maxsize=1)
def build_program():
    nc = bacc.Bacc("TRN2", target_bir_lowering=False, debug=False)

    x = nc.dram_tensor("x", [DIN * CI, PLANE], MM_DT, kind="ExternalInput").ap()
    wt = nc.dram_tensor("wt", [KP, 9 * CO], MM_DT, kind="ExternalInput").ap()
    b2 = nc.dram_tensor("b2", [2 * CO, 1], F32, kind="ExternalInput").ap()
    # y row (j*CO+co) = output plane 2t+j, channel co, position t*SPP+pos
    y = nc.dram_tensor("y", [2 * CO, NPAIR * SPP], F16, kind="ExternalOutput").ap()

    with tile.TileContext(nc) as tc:
        with (
            tc.tile_pool(name="wp", bufs=1) as wpool,
            tc.tile_pool(name="xp", bufs=3) as xpool,
            tc.tile_pool(name="op", bufs=3) as opool,
            tc.tile_pool(name="pa", bufs=3, space="PSUM") as papool,
            tc.tile_pool(name="pb", bufs=3, space="PSUM") as pbpool,
            tc.tile_pool(name="ps", bufs=1, space="PSUM") as pspool,
        ):
            # First window-pair input DMAs go out before anything else so the
            # PE's first real matmul isn't gated on the weights queue. The
            # loads are split at rows 12/30: the first slices cover only
            # chunk 0, so compute starts ~4us earlier; later slices land
            # while earlier chunks stream (their PE-side wait-absorbing
            # dummies sit between chunks — the PE queue is in-order).
            # First window-pair loads split at rows 12/30: the first slices
            # cover only chunk 0 so compute starts early; later slices land
            # while earlier chunks stream (their PE-side wait-absorbing
            # dummies sit between chunks — the PE queue is in-order). All on
            # the sync queue: scalar-queue DMAs start behind ACT_TABLE_LOAD.
            SP1 = 12 * DIN
            SP2 = 30 * DIN
            xe0 = xpool.tile([KP, PLANE], MM_DT, tag="xe", name="xe0")
            nc.sync.dma_start(xe0[:, :SP1], x[0:KP, :SP1])
            xo0 = xpool.tile([KP, PLANE], MM_DT, tag="xo", name="xo0")
            nc.sync.dma_start(xo0[:, :SP1], x[CI : CI + KP, :SP1])

            wa = wpool.tile([KP, 9 * CO], MM_DT)
            nc.sync.dma_start(wa[:, :], wt)
            bias_t = wpool.tile([2 * CO, 1], F32)
            nc.sync.dma_start(bias_t[:, :], b2)

            nc.sync.dma_start(xe0[:, SP1:SP2], x[0:KP, SP1:SP2])
            nc.sync.dma_start(xo0[:, SP1:SP2], x[CI : CI + KP, SP1:SP2])
            nc.sync.dma_start(xe0[:, SP2:], x[0:KP, SP2:])
            nc.sync.dma_start(xo0[:, SP2:], x[CI : CI + KP, SP2:])

            # Never-read scratch PSUM bank for "wait absorber" dummy matmuls.
            # A matmul's LDWEIGHTS uop has a single sync-wait slot, so each
            # real matmul may carry at most ONE semaphore wait. Dummies absorb
            # the DMA-completion waits (one dummy per freshly loaded tile) so
            # real matmuls only ever wait on their PSUM slot release.
            scr = pspool.tile([CO, 512], F32)

            # absorb the weights-DMA wait
            nc.tensor.matmul(
                scr[:, :2], wa[:, :CO], wa[:, :2], start=True, stop=True,
                tile_position=(0, 0),
            )

            for t in range(NPAIR):
                d0, d1 = 2 * t, 2 * t + 1
                # window tiles: planes d..d+2 stacked on partitions (kd,ci)
                if t == 0:
                    xe, xo = xe0, xo0
                else:
                    xe = xpool.tile([KP, PLANE], MM_DT, tag="xe", name=f"xe{t}")
                    nc.sync.dma_start(xe[:, :], x[CI * d0 : CI * d0 + KP, :])
                    xo = xpool.tile([KP, PLANE], MM_DT, tag="xo", name=f"xo{t}")
                    nc.sync.dma_start(xo[:, :], x[CI * d1 : CI * d1 + KP, :])
                xe3 = xe[:, :].rearrange("p (h w) -> p h w", w=DIN)
                xo3 = xo[:, :].rearrange("p (h w) -> p h w", w=DIN)

                # absorb the window-DMA waits (see scratch-bank comment above)
                nc.tensor.matmul(
                    scr[:, :2], wa[:, :CO], xe[:, :2], start=True, stop=True,
                    tile_position=(0, 0),
                )
                nc.tensor.matmul(
                    scr[:, :2], wa[:, :CO], xo[:, :2], start=True, stop=True,
                    tile_position=(0, 0),
                )

                ot = opool.tile([2 * CO, SPP], F16)

                chunks = CHUNKS_TAIL if t == NPAIR - 1 else CHUNKS
                for ci_, (h0, rows) in enumerate(chunks):
                    if t == 0 and ci_ in (1, 3):
                        # absorb the first pair's later load-slice waits here
                        # so earlier chunks aren't gated on them
                        hi = SP2 if ci_ == 1 else PLANE
                        nc.tensor.matmul(
                            scr[:, :2], wa[:, :CO], xe[:, hi - 2 : hi],
                            start=True, stop=True, tile_position=(0, 0),
                        )
                        nc.tensor.matmul(
                            scr[:, :2], wa[:, :CO], xo[:, hi - 2 : hi],
                            start=True, stop=True, tile_position=(0, 0),
                        )
                    n = rows * DOUT
                    # separate PSUM banks per col group: concurrent
                    # accumulation into one bank stalls the PE ~20%
                    pa_t = papool.tile([CO, 512], F32, tag="pa")
                    pb_t = pbpool.tile([2 * CO, 512], F32, tag="pb")
                    pa = pa_t[:, :n]
                    pb = pb_t[CO:, :n]
                    for kh in range(K):
                        for kw in range(K):
                            first = kh == 0 and kw == 0
                            last = kh == K - 1 and kw == K - 1
                            lhs = wa[:, (kh * K + kw) * CO : (kh * K + kw + 1) * CO]
                            rhe = xe3[:, h0 + kh : h0 + kh + rows, kw : kw + DOUT]
                            rho = xo3[:, h0 + kh : h0 + kh + rows, kw : kw + DOUT]
                            nc.tensor.matmul(
                                pa[:, :], lhs, rhe, start=first, stop=last,
                                tile_position=(0, 0),
                            )
                            nc.tensor.matmul(
                                pb[:, :], lhs, rho, start=first, stop=last,
                                tile_position=(0, CO),
                            )
                    cs = slice(h0 * DOUT, h0 * DOUT + n)
                    # drains: f32 PSUM + bias -> fp16 SBUF, one engine per plane
                    nc.scalar.activation(
                        ot[:CO, cs], pa[:, :],
                        mybir.ActivationFunctionType.Identity,
                        bias=bias_t[:CO, :],
                    )
                    nc.vector.tensor_scalar_add(ot[CO:, cs], pb[:, :], bias_t[CO:, :])
                    if t == NPAIR - 1:
                        # last pair: store per chunk to shorten the kernel
                        # tail; the final chunk goes out as two half-stores on
                        # the two HWDGE queues so neither waits for the other
                        # plane's drain engine
                        ys = slice(t * SPP + cs.start, t * SPP + cs.stop)
                        if ci_ == len(chunks) - 1:
                            nc.scalar.dma_start(y[:CO, ys], ot[:CO, cs])
                            nc.sync.dma_start(y[CO:, ys], ot[CO:, cs])
                        else:
                            nc.sync.dma_start(y[:, ys], ot[:, cs])
                if t < NPAIR - 1:
                    # one store per plane pair
                    nc.sync.dma_start(y[:, t * SPP : (t + 1) * SPP], ot[:, :])

    nc.compile()
    return nc


def make_in_maps(inputs, weight, bias):
    """Host-side shard/pack: returns per-core input maps."""
    inputs = np.ascontiguousarray(np.asarray(inputs, dtype=np.float32))
    weight = np.asarray(weight, dtype=np.float32)
    bias = np.asarray(bias, dtype=np.float32)
    # weights: [(kd,ci), (kh,kw,co)]
    wt = _pack_mm(weight.transpose(2, 1, 3, 4, 0).reshape(KP, 9 * CO))
    b2 = np.ascontiguousarray(np.tile(bias, 2).reshape(2 * CO, 1))
    in_maps = []
    for c in range(B):
        xc = _pack_mm(inputs[c].transpose(1, 0, 2, 3).reshape(DIN * CI, PLANE))
        in_maps.append({"x": xc, "wt": wt, "b2": b2})
    return in_maps


def kernel(inputs, weight, bias, **run_kwargs):
    nc = build_program()
    in_maps = make_in_maps(inputs, weight, bias)
    res = run_bass_kernel_spmd(nc, in_maps, core_ids=list(range(B)), **run_kwargs)
    out = np.empty((B, CO, DOUT, DOUT, DOUT), dtype=np.float32)
    for c in range(B):
        yv = res.results[c]["y"].astype(np.float32).reshape(2, CO, NPAIR, DOUT, DOUT)
        out[c, :, 0::2] = yv[0]
        out[c, :, 1::2] = yv[1]
    return out


# revision 25
# speedup vs baseline: 1.0034x; 1.0034x over previous
"""Conv3d (8,32,48,48,48) * (64,32,3,3,3) -> (8,64,46,46,46), valid, stride 1.

Strategy: data-parallel over batch (1 image per NeuronCore, 8 cores).
Per core the conv is an implicit GEMM:
  out[co, d, h', w'] = sum_{kd,ci,kh,kw} W[co,ci,kd,kh,kw] * X[ci, d+kd, h'+kh, w'+kw]
- contraction K = (kd, ci) = 3*32 = 96 partitions: a SBUF "window" tile
  holds input planes d..d+2 stacked on partitions (plane-major),
- kh, kw are pure free-dim offsets into the window tile (rows step 48),
  so each output tile is 9 accumulating matmuls into one PSUM bank,
- two output planes per window iteration, col-tiled on the PE
  (even plane -> PE col group h0, odd -> col group h64) so the two
  46x46-position streams run concurrently at 1 col/cycle each,
- each col group accumulates into its OWN PSUM bank (sharing one bank
  across groups costs ~20% PE stall from bank write contention),
- drains: ScalarE activation (even) / VectorE tensor_scalar (odd),
  f32 PSUM + bias -> fp16 SBUF,
- output staged as fp16 [2*CO, 23*SPP] (row = (parity, co)); one DMA
  per plane-pair; host reinterleaves parities and upcasts to f32.
"""

import functools
import os

import numpy as np

import concourse.bacc as bacc
import concourse.tile as tile
from concourse import mybir
from concourse.bass_utils import run_bass_kernel_spmd

# Problem constants (hardcoded per harness contract)
B = 8
CI = 32
DIN = 48
CO = 64
K = 3
DOUT = DIN - K + 1  # 46
SPP = DOUT * DOUT  # 2116 spatial positions per output plane
PLANE = DIN * DIN  # 2304 elements per (ci, plane)
KP = K * CI  # 96 contraction partitions
NPAIR = DOUT // 2  # 23 plane pairs

# h'-row chunking of a 46x46 output plane into PSUM-bank-sized matmuls
CHUNKS = [(0, 10), (10, 9), (19, 9), (28, 9), (37, 9)]  # (h0, rows) -> N = rows*46
# last pair: finer trailing chunks so the final drain+store tail is shorter
CHUNKS_TAIL = [(0, 10), (10, 9), (19, 9), (28, 9), (37, 5), (42, 4)]

F32 = mybir.dt.float32
F16 = mybir.dt.float16

MODE = os.environ.get("CONV_MM_DT", "fp16")
MM_DT = mybir.dt.bfloat16 if MODE == "bf16" else F16


def _pack_mm(a):
    """Host-side cast of a float32 array to the matmul operand format."""
    if MODE == "bf16":
        import ml_dtypes

        return np.ascontiguousarray(a.astype(ml_dtypes.bfloat16))
    return np.ascontiguousarray(a.astype(np.float16))


@functools.lru_cache(maxsize=1)
def build_program():
    nc = bacc.Bacc("TRN2", target_bir_lowering=False, debug=False)

    x = nc.dram_tensor("x", [DIN * CI, PLANE], MM_DT, kind="ExternalInput").ap()
    wt = nc.dram_tensor("wt", [KP, 9 * CO], MM_DT, kind="ExternalInput").ap()
    b2 = nc.dram_tensor("b2", [2 * CO, 1], F32, kind="ExternalInput").ap()
    # y row (j*CO+co) = output plane 2t+j, channel co, position t*SPP+pos
    y = nc.dram_tensor("y", [2 * CO, NPAIR * SPP], F16, kind="ExternalOutput").ap()

    with tile.TileContext(nc) as tc:
        with (
            tc.tile_pool(name="wp", bufs=1) as wpool,
            tc.tile_pool(name="xp", bufs=3) as xpool,
            tc.tile_pool(name="op", bufs=3) as opool,
            tc.tile_pool(name="pa", bufs=3, space="PSUM") as papool,
            tc.tile_pool(name="pb", bufs=3, space="PSUM") as pbpool,
            tc.tile_pool(name="ps", bufs=1, space="PSUM") as pspool,
        ):
            # First window-pair input DMAs go out before the weights so the
            # PE's first real matmul isn't gated on the weights queue. The
            # loads are split at rows 12/30: the first slices cover only
            # chunk 0, so compute starts ~2us earlier; later slices land
            # while earlier chunks stream (their PE-side wait-absorbing
            # dummies sit between chunks — the PE queue is in-order). All on
            # the sync queue: scalar-queue DMAs start behind ACT_TABLE_LOAD.
            SP1 = 12 * DIN
            SP2 = 30 * DIN
            xe0 = xpool.tile([KP, PLANE], MM_DT, tag="xe", name="xe0")
            nc.sync.dma_start(xe0[:, :SP1], x[0:KP, :SP1])
            xo0 = xpool.tile([KP, PLANE], MM_DT, tag="xo", name="xo0")
            nc.sync.dma_start(xo0[:, :SP1], x[CI : CI + KP, :SP1])

            wa = wpool.tile([KP, 9 * CO], MM_DT)
            nc.sync.dma_start(wa[:, :], wt)
            bias_t = wpool.tile([2 * CO, 1], F32)
            nc.sync.dma_start(bias_t[:, :], b2)

            nc.sync.dma_start(xe0[:, SP1:SP2], x[0:KP, SP1:SP2])
            nc.sync.dma_start(xo0[:, SP1:SP2], x[CI : CI + KP, SP1:SP2])
            nc.sync.dma_start(xe0[:, SP2:], x[0:KP, SP2:])
            nc.sync.dma_start(xo0[:, SP2:], x[CI : CI + KP, SP2:])

            # Never-read scratch PSUM bank for "wait absorber" dummy matmuls.
            # A matmul's LDWEIGHTS uop has a single sync-wait slot, so each
            # real matmul may carry at most ONE semaphore wait. Dummies absorb
            # the DMA-completion waits (one dummy per freshly loaded tile) so
            # real matmuls only ever wait on their PSUM slot release.
            scr = pspool.tile([CO, 512], F32)

            # absorb the weights-DMA wait
            nc.tensor.matmul(
                scr[:, :2], wa[:, :CO], wa[:, :2], start=True, stop=True,
                tile_position=(0, 0),
            )

            for t in range(NPAIR):
                d0, d1 = 2 * t, 2 * t + 1
                # window tiles: planes d..d+2 stacked on partitions (kd,ci)
                if t == 0:
                    xe, xo = xe0, xo0
                else:
                    xe = xpool.tile([KP, PLANE], MM_DT, tag="xe", name=f"xe{t}")
                    nc.sync.dma_start(xe[:, :], x[CI * d0 : CI * d0 + KP, :])
                    xo = xpool.tile([KP, PLANE], MM_DT, tag="xo", name=f"xo{t}")
                    nc.sync.dma_start(xo[:, :], x[CI * d1 : CI * d1 + KP, :])
                xe3 = xe[:, :].rearrange("p (h w) -> p h w", w=DIN)
                xo3 = xo[:, :].rearrange("p (h w) -> p h w", w=DIN)

                # absorb the window-DMA waits (see scratch-bank comment above)
                nc.tensor.matmul(
                    scr[:, :2], wa[:, :CO], xe[:, :2], start=True, stop=True,
                    tile_position=(0, 0),
                )
                nc.tensor.matmul(
                    scr[:, :2], wa[:, :CO], xo[:, :2], start=True, stop=True,
                    tile_position=(0, 0),
                )

                ot = opool.tile([2 * CO, SPP], F16)

                chunks = CHUNKS_TAIL if t == NPAIR - 1 else CHUNKS
                for ci_, (h0, rows) in enumerate(chunks):
                    if t == 0 and ci_ in (1, 3):
                        # absorb the first pair's later load-slice waits here
                        # so earlier chunks aren't gated on them
                        hi = SP2 if ci_ == 1 else PLANE
                        nc.tensor.matmul(
                            scr[:, :2], wa[:, :CO], xe[:, hi - 2 : hi],
                            start=True, stop=True, tile_position=(0, 0),
                        )
                        nc.tensor.matmul(
                            scr[:, :2], wa[:, :CO], xo[:, hi - 2 : hi],
                            start=True, stop=True, tile_position=(0, 0),
                        )
                    n = rows * DOUT
                    # separate PSUM banks per col group: concurrent
                    # accumulation into one bank stalls the PE ~20%
                    pa_t = papool.tile([CO, 512], F32, tag="pa")
                    pb_t = pbpool.tile([2 * CO, 512], F32, tag="pb")
                    pa = pa_t[:, :n]
                    pb = pb_t[CO:, :n]
                    for kh in range(K):
                        for kw in range(K):
                            first = kh == 0 and kw == 0
                            last = kh == K - 1 and kw == K - 1
                            lhs = wa[:, (kh * K + kw) * CO : (kh * K + kw + 1) * CO]
                            rhe = xe3[:, h0 + kh : h0 + kh + rows, kw : kw + DOUT]
                            rho = xo3[:, h0 + kh : h0 + kh + rows, kw : kw + DOUT]
                            nc.tensor.matmul(
                                pa[:, :], lhs, rhe, start=first, stop=last,
                                tile_position=(0, 0),
                            )
                            nc.tensor.matmul(
                                pb[:, :], lhs, rho, start=first, stop=last,
                                tile_position=(0, CO),
                            )
                    cs = slice(h0 * DOUT, h0 * DOUT + n)
                    # drains: f32 PSUM + bias -> fp16 SBUF, one engine per plane
                    nc.scalar.activation(
                        ot[:CO, cs], pa[:, :],
                        mybir.ActivationFunctionType.Identity,
                        bias=bias_t[:CO, :],
                    )
                    nc.vector.tensor_scalar_add(ot[CO:, cs], pb[:, :], bias_t[CO:, :])
                    if t == NPAIR - 1:
                        # last pair: store per chunk to shorten the kernel tail
                        nc.sync.dma_start(
                            y[:, t * SPP + cs.start : t * SPP + cs.stop], ot[:, cs]
                        )
                if t < NPAIR - 1:
                    # one store per plane pair
                    nc.sync.dma_start(y[:, t * SPP : (t + 1) * SPP], ot[:, :])

    nc.compile()
    return nc


def make_in_maps(inputs, weight, bias):
    """Host-side shard/pack: returns per-core input maps."""
    inputs = np.ascontiguousarray(np.asarray(inputs, dtype=np.float32))
    weight = np.asarray(weight, dtype=np.float32)
    bias = np.asarray(bias, dtype=np.float32)
    # weights: [(kd,ci), (kh,kw,co)]
    wt = _pack_mm(weight.transpose(2, 1, 3, 4, 0).reshape(KP, 9 * CO))
    b2 = np.ascontiguousarray(np.tile(bias, 2).reshape(2 * CO, 1))
    in_maps = []
    for c in range(B):
        xc = _pack_mm(inputs[c].transpose(1, 0, 2, 3).reshape(DIN * CI, PLANE))
        in_maps.append({"x": xc, "wt": wt, "b2": b2})
    return in_maps


def kernel(inputs, weight, bias, **run_kwargs):
    nc = build_program()
    in_maps = make_in_maps(inputs, weight, bias)
    res = run_bass_kernel_spmd(nc, in_maps, core_ids=list(range(B)), **run_kwargs)
    out = np.empty((B, CO, DOUT, DOUT, DOUT), dtype=np.float32)
    for c in range(B):
        yv = res.results[c]["y"].astype(np.float32).reshape(2, CO, NPAIR, DOUT, DOUT)
        out[c, :, 0::2] = yv[0]
        out[c, :, 1::2] = yv[1]
    return out

